# revision 11
# baseline (speedup 1.0000x reference)
"""AttentionLSTM Trainium2 kernel — 8-core data-parallel.

Model (per batch row b): two independent single-direction LSTMs over T=43
steps of x[:, :, t] (H=300 features), hidden states summed, then a
conv-softmax attention over time, tanh, fc(300->80), softmax.

Device mapping per core (512 batch rows):
  - z^T[1200, 512] per (direction, step) via PE matmuls with K padded
    300->384 (3 k-tiles of 128), M gate-aligned tiles {128,128,44}.
  - MM inputs in 16-bit (fp16 default) at 1 cycle/row; accumulation fp32.
  - gates: ScalarE sigmoid/tanh with fused per-partition bias, VectorE
    fused [sig_i|sig_f] * [tanh_g|c] products, c/h state in SBUF.
  - attention accumulated online: e_t = sigmoid(a)/(1-sigmoid(a)) = exp(a)
    (avoids exp table loads mid-loop); r += hsum_t * e_t on GPSIMD.
  - tail: hStar = tanh(r/s), logits = fc(hStar) via PE (batch on PSUM
    partitions), softmax over the 80-class free dim.
"""

import os
import sys

sys.path.insert(0, "/opt/trn_rl_repo")

from contextlib import ExitStack

import numpy as np

import concourse.bass as bass
import concourse.tile as tile
from concourse import mybir
from concourse.bass_utils import run_bass_kernel_spmd

f32 = mybir.dt.float32
AF = mybir.ActivationFunctionType
AX = mybir.AxisListType

_BIRFIX_DONE = False


def _split_multiwaits(bir_json):
    """This walrus build allows one sync-wait per engine instruction; Tile
    attaches one per producer proc. Hoist extras onto standalone
    EventSemaphore instructions inserted just before, same engine queue."""
    import json
    j = json.loads(bir_json.decode() if isinstance(bir_json, bytes) else bir_json)
    for fn in j.get("functions", []):
        for blk in fn.get("blocks", []):
            out = []
            for ins in blk.get("instructions", []):
                si = ins.get("sync_info")
                ow = si.get("on_wait") if si else None
                if ow and len(ow) > 1:
                    for i, w in enumerate(ow[:-1]):
                        out.append({
                            "debug": ins.get("debug", 0),
                            "engine": ins["engine"],
                            "ins": [], "outs": [],
                            "name": f"{ins['name']}_xw{i}",
                            "opcode": "EventSemaphore",
                            "sync_info": {"on_update": [], "on_wait": [w]},
                        })
                    si["on_wait"] = [ow[-1]]
                out.append(ins)
            blk["instructions"] = out
    return json.dumps(j).encode()


def _install_birfix():
    global _BIRFIX_DONE
    if _BIRFIX_DONE:
        return
    from concourse import bass2jax
    orig = bass2jax.compile_bir_kernel

    def patched(bir_json, tmpdir, neff_name="file.neff"):
        return orig(_split_multiwaits(bir_json), tmpdir, neff_name)

    bass2jax.compile_bir_kernel = patched
    _BIRFIX_DONE = True


class _Runner:
    """Compile once; keep the sharded jitted executable + device inputs."""

    def __init__(self, nc, n_cores):
        import jax
        from jax.sharding import Mesh, PartitionSpec
        from jax.experimental.shard_map import shard_map
        from concourse import bass2jax as b2j

        b2j.install_neuronx_cc_hook()
        _install_birfix()
        self.jax = jax
        self.nc = nc
        self.n_cores = n_cores
        part_name = nc.partition_id_tensor.name if nc.partition_id_tensor else None
        in_names, out_names, out_avals, zero_outs = [], [], [], []
        for alloc in nc.m.functions[0].allocations:
            if not isinstance(alloc, mybir.MemoryLocationSet):
                continue
            name = alloc.memorylocations[0].name
            if alloc.kind == "ExternalInput":
                if name != part_name:
                    in_names.append(name)
            elif alloc.kind == "ExternalOutput":
                out_names.append(name)
                shape = tuple(alloc.tensor_shape)
                dtype = mybir.dt.np(alloc.dtype)
                out_avals.append(jax.core.ShapedArray(shape, dtype))
                zero_outs.append(np.zeros(shape, dtype))
        self.in_names = list(in_names)
        self.out_names = out_names
        self.out_avals = out_avals
        self.zero_outs = zero_outs
        n_params = len(in_names)
        n_outs = len(out_avals)
        all_names = in_names + out_names
        if part_name is not None:
            all_names = all_names + [part_name]
        donate = tuple(range(n_params, n_params + n_outs))

        def _body(*args):
            operands = list(args)
            if part_name is not None:
                operands.append(b2j.partition_id_tensor())
            outs = b2j._bass_exec_p.bind(
                *operands,
                out_avals=tuple(out_avals),
                in_names=tuple(all_names),
                out_names=tuple(out_names),
                lowering_input_output_aliases=(),
                sim_require_finite=True,
                sim_require_nnan=True,
                nc=nc,
            )
            return tuple(outs)

        devices = jax.devices()[:n_cores]
        self.mesh = Mesh(np.asarray(devices), ("core",))
        in_specs = (PartitionSpec("core"),) * (n_params + n_outs)
        out_specs = (PartitionSpec("core"),) * n_outs
        self.sharded = jax.jit(
            shard_map(_body, mesh=self.mesh, in_specs=in_specs,
                      out_specs=out_specs, check_rep=False),
            donate_argnums=donate, keep_unused=True)
        self.sharding = jax.sharding.NamedSharding(
            self.mesh, PartitionSpec("core"))

    def put_inputs(self, in_maps):
        jax = self.jax
        concat = [np.concatenate([np.asarray(m[n]) for m in in_maps], axis=0)
                  for n in self.in_names]
        return [jax.device_put(a, self.sharding) for a in concat]

    def call(self, dev_in):
        zeros = [np.zeros((self.n_cores * z.shape[0], *z.shape[1:]), z.dtype)
                 for z in self.zero_outs]
        outs = self.sharded(*dev_in, *zeros)
        self.jax.block_until_ready(outs)
        return outs

    def run(self, in_maps):
        dev_in = self.put_inputs(in_maps)
        outs = self.call(dev_in)
        n = self.n_cores
        return [
            {name: np.asarray(outs[i]).reshape(n, *self.out_avals[i].shape)[c]
             for i, name in enumerate(self.out_names)}
            for c in range(n)
        ]

    def bench(self, in_maps, iters=5):
        import time
        dev_in = self.put_inputs(in_maps)
        self.call(dev_in)  # warm
        times = []
        for _ in range(iters):
            t0 = time.perf_counter()
            self.call(dev_in)
            times.append(time.perf_counter() - t0)
        return times

B, H, T, NCLS = 4096, 300, 43, 80
NCORES = 8
BS = B // NCORES          # 512 batch rows per core
NK = 3                    # k tiles (K padded 300 -> 384)
HP = NK * 128             # 384
MT = [(0, 128), (128, 128), (256, 44)]   # gate-aligned M tiles
GATES = [("i", 0), ("f", 300), ("g", 600), ("o", 900)]  # torch order i,f,g,o

MM_DT_NAME = os.environ.get("LSTM_MM_DT", "float16")
TRACE = False
LAST_EXEC_NS = None

_CACHE = {}


def _build(mdt_name):
    mdt = getattr(mybir.dt, mdt_name)
    nc = bass.Bass(target_bir_lowering=False)

    xt_d = nc.declare_dram_parameter("xt", [T, NK, 128, BS], mdt, isOutput=False)
    wih_d = nc.declare_dram_parameter("wih", [2, NK, 128, 1200], mdt, isOutput=False)
    whh_d = nc.declare_dram_parameter("whh", [2, NK, 128, 1200], mdt, isOutput=False)
    bias_d = nc.declare_dram_parameter("biasp", [128, 24], f32, isOutput=False)
    conv_d = nc.declare_dram_parameter("convp", [128, NK], mdt, isOutput=False)
    fcw_d = nc.declare_dram_parameter("fcw", [128, NK * NCLS], mdt, isOutput=False)
    fcb_d = nc.declare_dram_parameter("fcb", [1, NCLS], mdt, isOutput=False)
    out_d = nc.declare_dram_parameter("out", [BS, NCLS], f32, isOutput=True)

    with tile.TileContext(nc) as tc, ExitStack() as ctx:
        P = lambda name, bufs, **kw: ctx.enter_context(
            tc.tile_pool(name=name, bufs=bufs, **kw))
        wpool = P("w", 1)
        xpool = P("x", 2)
        zpool = P("z", 6, space="PSUM")
        apool = P("aps", 1, space="PSUM")
        ebpp = P("ebps", 1, space="PSUM")
        sifp = P("sif", 3)
        sop = P("so", 3)
        gcp = P("gc", 1)
        p1p = P("p1", 2)
        tcp = P("tc", 2)
        hp = P("h", 1)
        hsp = P("hs", 2)
        thp = P("th", 2)
        rp = P("r", 1)
        smp = P("sm", 2)
        ebp = P("eb", 2)
        fin = P("fin", 2)

        # ---- weights / constants ----
        wih_sb = wpool.tile([128, 2 * NK * 1200], mdt, tag="wih")
        whh_sb = wpool.tile([128, 2 * NK * 1200], mdt, tag="whh")
        for d in range(2):
            for k in range(NK):
                c0 = (d * NK + k) * 1200
                nc.sync.dma_start(out=wih_sb[:, c0:c0 + 1200], in_=wih_d.ap()[d, k])
                nc.sync.dma_start(out=whh_sb[:, c0:c0 + 1200], in_=whh_d.ap()[d, k])
        bias_sb = wpool.tile([128, 24], f32, tag="bias")
        nc.sync.dma_start(out=bias_sb, in_=bias_d.ap())
        conv_sb = wpool.tile([128, NK], mdt, tag="conv")
        nc.sync.dma_start(out=conv_sb, in_=conv_d.ap())
        fcw_sb = wpool.tile([128, NK * NCLS], mdt, tag="fcw")
        nc.sync.dma_start(out=fcw_sb, in_=fcw_d.ap())
        fcb_sb = wpool.tile([1, NCLS], mdt, tag="fcb")
        nc.sync.dma_start(out=fcb_sb, in_=fcb_d.ap())
        ones_sb = wpool.tile([1, 128], mdt, tag="ones")
        nc.vector.memset(ones_sb, 1.0)

        # ---- persistent state ----
        h = {}     # h[d][j]: [128, BS] mdt   (j == k-tile of next step's rhs)
        gc = {}    # gc[(d, j)]: [128, 1024] f32 = [tanh_g | c]
        for d in range(2):
            h[d] = []
            for j in range(NK):
                ht = hp.tile([128, BS], mdt, tag=f"h_{d}_{j}")
                nc.vector.memset(ht, 0.0)
                h[d].append(ht)
                g = gcp.tile([128, 1024], f32, tag=f"gc_{d}_{j}")
                nc.vector.memset(g, 0.0)
                gc[(d, j)] = g
        r = []
        for j in range(NK):
            rt = rp.tile([128, BS], f32, tag=f"r_{j}")
            nc.vector.memset(rt, 0.0)
            r.append(rt)
        ssum = rp.tile([1, BS], f32, tag="ssum")
        nc.vector.memset(ssum, 0.0)

        def w_slice(wsb, d, k, grow0, moff, msz):
            c0 = (d * NK + k) * 1200 + grow0 + moff
            return wsb[:, c0:c0 + msz]

        # ---- time loop ----
        for t in range(T):
            xk = []
            for k in range(NK):
                xkt = xpool.tile([128, BS], mdt, tag=f"x{k}")
                nc.sync.dma_start(out=xkt, in_=xt_d.ap()[t, k])
                xk.append(xkt)

            for d in range(2):
                for j, (moff, msz) in enumerate(MT):
                    sif = sifp.tile([128, 1024], f32, tag="sif")
                    so = sop.tile([128, BS], f32, tag="so")
                    gcj = gc[(d, j)]
                    for gi, (gname, grow0) in enumerate(GATES):
                        zp = zpool.tile([128, BS], f32, tag="z")
                        zs = zp[:msz]
                        for k in range(NK):
                            nc.tensor.matmul(
                                zs, lhsT=w_slice(wih_sb, d, k, grow0, moff, msz),
                                rhs=xk[k], start=(k == 0), stop=False)
                        for k in range(NK):
                            nc.tensor.matmul(
                                zs, lhsT=w_slice(whh_sb, d, k, grow0, moff, msz),
                                rhs=h[d][k], start=False, stop=(k == NK - 1))
                        bcol = d * 12 + gi * 3 + j
                        bap = bias_sb[:msz, bcol:bcol + 1]
                        if gname == "g":
                            nc.scalar.activation(out=gcj[:msz, 0:512], in_=zs,
                                                 func=AF.Tanh, bias=bap)
                        elif gname == "i":
                            nc.scalar.activation(out=sif[:msz, 0:512], in_=zs,
                                                 func=AF.Sigmoid, bias=bap)
                        elif gname == "f":
                            nc.scalar.activation(out=sif[:msz, 512:1024], in_=zs,
                                                 func=AF.Sigmoid, bias=bap)
                        else:
                            nc.scalar.activation(out=so[:msz], in_=zs,
                                                 func=AF.Sigmoid, bias=bap)
                    # c_new = sig_f * c + sig_i * tanh_g ; h = sig_o * tanh(c_new)
                    p1 = p1p.tile([128, 1024], f32, tag="p1")
                    nc.vector.tensor_mul(out=p1[:msz], in0=sif[:msz], in1=gcj[:msz])
                    nc.vector.tensor_add(out=gcj[:msz, 512:1024],
                                         in0=p1[:msz, 0:512], in1=p1[:msz, 512:1024])
                    tcj = tcp.tile([128, BS], f32, tag="tc")
                    nc.scalar.activation(out=tcj[:msz], in_=gcj[:msz, 512:1024],
                                         func=AF.Tanh)
                    nc.vector.tensor_mul(out=h[d][j][:msz], in0=so[:msz],
                                         in1=tcj[:msz])

            # ---- attention (online, unnormalized softmax) ----
            hs, th = [], []
            for j in range(NK):
                hsj = hsp.tile([128, BS], f32, tag=f"hs{j}")
                nc.gpsimd.tensor_add(out=hsj, in0=h[0][j], in1=h[1][j])
                hs.append(hsj)
                thj = thp.tile([128, BS], mdt, tag=f"th{j}")
                nc.scalar.activation(out=thj, in_=hsj, func=AF.Tanh)
                th.append(thj)
            a_ps = apool.tile([1, BS], f32, tag="a")
            for k in range(NK):
                nc.tensor.matmul(a_ps, lhsT=conv_sb[:, k:k + 1], rhs=th[k],
                                 start=(k == 0), stop=(k == NK - 1))
            sg = smp.tile([1, BS], f32, tag="sg")
            nc.scalar.activation(out=sg, in_=a_ps, func=AF.Sigmoid)
            om = smp.tile([1, BS], f32, tag="om")
            nc.scalar.activation(out=om, in_=sg, func=AF.Copy, bias=1.0, scale=-1.0)
            nc.vector.reciprocal(out=om, in_=om)
            e = smp.tile([1, BS], mdt, tag="e")
            nc.vector.tensor_mul(out=e, in0=sg, in1=om)   # e = exp(a), rounded
            eb_ps = ebpp.tile([128, BS], f32, tag="ebp")
            nc.tensor.matmul(eb_ps, lhsT=ones_sb, rhs=e, start=True, stop=True)
            eb = ebp.tile([128, BS], f32, tag="eb")
            nc.scalar.activation(out=eb, in_=eb_ps, func=AF.Copy)
            # s accumulates the same rounded e as r, so rounding cancels
            nc.vector.tensor_add(out=ssum, in0=ssum, in1=eb[0:1])
            for j in range(NK):
                tmp = ebp.tile([128, BS], f32, tag="rt")
                nc.gpsimd.tensor_mul(out=tmp, in0=hs[j], in1=eb)
                nc.gpsimd.tensor_add(out=r[j], in0=r[j], in1=tmp)

        # ---- tail: hStar = tanh(r / s); logits; softmax ----
        rs = smp.tile([1, BS], f32, tag="rs")
        nc.vector.reciprocal(out=rs, in_=ssum)
        rs16 = smp.tile([1, BS], mdt, tag="rs16")
        nc.scalar.activation(out=rs16, in_=rs, func=AF.Copy)
        rsb = ebpp.tile([128, BS], f32, tag="ebp")
        nc.tensor.matmul(rsb, lhsT=ones_sb, rhs=rs16, start=True, stop=True)
        hst = []
        for j in range(NK):
            hn = fin.tile([128, BS], f32, tag=f"hn{j}")
            nc.vector.tensor_mul(out=hn, in0=r[j], in1=rsb)
            hj = fin.tile([128, BS], mdt, tag=f"hst{j}")
            nc.scalar.activation(out=hj, in_=hn, func=AF.Tanh)
            hst.append(hj)
        for bt in range(BS // 128):
            fcp = apool.tile([128, NCLS], f32, tag="a")
            for j in range(NK):
                nc.tensor.matmul(fcp, lhsT=hst[j][:, bt * 128:(bt + 1) * 128],
                                 rhs=fcw_sb[:, j * NCLS:(j + 1) * NCLS],
                                 start=(j == 0), stop=False)
            nc.tensor.matmul(fcp, lhsT=ones_sb, rhs=fcb_sb, start=False, stop=True)
            mx = fin.tile([128, 1], f32, tag="mx")
            nc.vector.reduce_max(out=mx, in_=fcp, axis=AX.X)
            nmx = fin.tile([128, 1], f32, tag="nmx")
            nc.vector.tensor_scalar_mul(out=nmx, in0=mx, scalar1=-1.0)
            ex = fin.tile([128, NCLS], f32, tag="ex")
            nc.scalar.activation(out=ex, in_=fcp, func=AF.Exp, bias=nmx)
            sm = fin.tile([128, 1], f32, tag="smm")
            nc.vector.reduce_sum(out=sm, in_=ex, axis=AX.X)
            nc.vector.reciprocal(out=sm, in_=sm)
            ot = fin.tile([128, NCLS], f32, tag="ot")
            nc.vector.tensor_scalar_mul(out=ot, in0=ex, scalar1=sm)
            nc.sync.dma_start(out=out_d.ap()[bt * 128:(bt + 1) * 128], in_=ot)

    return nc


def _prep(x, w_ih, w_hh, b_ih, b_hh, conv_w, fc_w, fc_b, np_mdt):
    """Host-side layout prep (shared across cores + per-core x shards)."""
    wih = np.zeros((2, NK, 128, 1200), np.float32)
    whh = np.zeros((2, NK, 128, 1200), np.float32)
    for d in range(2):
        wt_i = w_ih[d].T  # [300, 1200]
        wt_h = w_hh[d].T
        for k in range(NK):
            r0, r1 = k * 128, min((k + 1) * 128, H)
            if r0 < H:
                wih[d, k, :r1 - r0] = wt_i[r0:r1]
                whh[d, k, :r1 - r0] = wt_h[r0:r1]
    bias = (b_ih + b_hh).astype(np.float32)  # [2, 1200]
    biasp = np.zeros((128, 24), np.float32)
    for d in range(2):
        for gi, (_, grow0) in enumerate(GATES):
            for j, (moff, msz) in enumerate(MT):
                biasp[:msz, d * 12 + gi * 3 + j] = bias[d, grow0 + moff:grow0 + moff + msz]
    convp = np.zeros((128, NK), np.float32)
    for k in range(NK):
        r0, r1 = k * 128, min((k + 1) * 128, H)
        if r0 < H:
            convp[:r1 - r0, k] = conv_w[r0:r1]
    fcw = np.zeros((128, NK * NCLS), np.float32)
    fwT = fc_w.T  # [300, 80]
    for k in range(NK):
        r0, r1 = k * 128, min((k + 1) * 128, H)
        if r0 < H:
            fcw[:r1 - r0, k * NCLS:(k + 1) * NCLS] = fwT[r0:r1]

    shared = {
        "wih": wih.astype(np_mdt),
        "whh": whh.astype(np_mdt),
        "biasp": biasp,
        "convp": convp.astype(np_mdt),
        "fcw": fcw.astype(np_mdt),
        "fcb": fc_b.reshape(1, NCLS).astype(np_mdt),
    }

    # x: [B, H, T] -> per-core [T, NK, 128, BS] (zero-padded H)
    xs = np.ascontiguousarray(np.transpose(x, (2, 1, 0)))  # [T, H, B]
    xp = np.zeros((T, HP, B), np.float32)
    xp[:, :H] = xs
    xp = xp.reshape(T, NK, 128, NCORES, BS)
    in_maps = []
    for c in range(NCORES):
        m = dict(shared)
        m["xt"] = np.ascontiguousarray(xp[:, :, :, c]).astype(np_mdt)
        in_maps.append(m)
    return in_maps


def kernel(x, w_ih, w_hh, b_ih, b_hh, conv_w, fc_w, fc_b):
    global LAST_EXEC_NS
    mdt_name = MM_DT_NAME
    np_mdt = np.float16 if mdt_name == "float16" else (
        __import__("ml_dtypes").bfloat16 if mdt_name == "bfloat16" else np.float32)
    if mdt_name not in _CACHE:
        _CACHE[mdt_name] = _Runner(_build(mdt_name), NCORES)
    runner = _CACHE[mdt_name]
    in_maps = _prep(np.asarray(x, np.float32), np.asarray(w_ih, np.float32),
                    np.asarray(w_hh, np.float32), np.asarray(b_ih, np.float32),
                    np.asarray(b_hh, np.float32), np.asarray(conv_w, np.float32),
                    np.asarray(fc_w, np.float32), np.asarray(fc_b, np.float32),
                    np_mdt)
    results = runner.run(in_maps)
    out = np.concatenate([r["out"] for r in results], axis=0)
    return out.astype(np.float32)


def bench(x, w_ih, w_hh, b_ih, b_hh, conv_w, fc_w, fc_b, iters=5):
    mdt_name = MM_DT_NAME
    np_mdt = np.float16 if mdt_name == "float16" else (
        __import__("ml_dtypes").bfloat16 if mdt_name == "bfloat16" else np.float32)
    if mdt_name not in _CACHE:
        _CACHE[mdt_name] = _Runner(_build(mdt_name), NCORES)
    runner = _CACHE[mdt_name]
    in_maps = _prep(np.asarray(x, np.float32), np.asarray(w_ih, np.float32),
                    np.asarray(w_hh, np.float32), np.asarray(b_ih, np.float32),
                    np.asarray(b_hh, np.float32), np.asarray(conv_w, np.float32),
                    np.asarray(fc_w, np.float32), np.asarray(fc_b, np.float32),
                    np_mdt)
    return runner.bench(in_maps, iters=iters)
